# revision 37
# baseline (speedup 1.0000x reference)
"""Multi-head attention (B=2, S=2048, E=1024, H=16) on 8 TRN2 NeuronCores.

Sharding: batch x head-group. Core c handles batch b=c//4 and head group
g=c%4 (4 heads = 256 of E). Each core computes its heads' attention output
slice and a partial fc_out product [S, E]; the host sums the 4 partials per
batch and adds b_out.

v2 design notes (vs the 239us baseline):
- All HBM input tensors are f16 (host-cast); qpT/kpT kept on-chip in f32r.
- Single fused [S, E] f16 output per core: fc_out contracts K=256 over both
  head-pairs (2 accumulating matmuls) -> half the output DMA and copies.
- Loop order: outer qb (512-query block), inner pt (head pair). Normalize
  and fc_out are software-pipelined one unit behind attention so the PE
  instruction stream never waits on the recip chain (keeps PE HAM-warm).
- Act engine runs ONLY exp, as [128, 2, 512] pair-instructions spanning the
  two score psum banks. Copies are on gpsimd, recip/normalize-mult on DVE.
- Softmax denominator: ones-column trick in the AV matmul (row 64 of the
  [65, 512] psum); per-query reciprocal row is partition-broadcast with a
  single K=2 matmul against a constant [2, 128] "eye64" block matrix.
- K/V projections are chunked at 128 keys to pace with their DMAs, so the
  PE starts working ~4us in and stays continuously busy.

Mask handling is exact: masked K/V rows are removed on the host (gather),
so softmax(where(mask==0, -1e20, e)) == exp(e_valid)/sum(exp(e_valid)).
"""

import os

import numpy as np

B, S, E, H = 2, 2048, 1024, 16
D = E // H           # 64
NCORES = 8
GROUPS = 4           # head groups per batch (cores per batch)
HPG = H // GROUPS    # 4 heads per core
DC = E // GROUPS     # 256 dims per core
NB = E // 128        # 8 contraction chunks over E
QB = 512             # query block width
NQB = S // QB        # 4

_CACHE = {}


def _split_excess_waits(nc, max_waits=1):
    """walrus rejects instructions carrying >1 sem wait; spread extras onto
    single-wait NoOps inserted before the instruction on the same engine."""
    import concourse.mybir as mybir

    n_split = 0
    for f in nc.m.functions:
        for bb in f.blocks:
            out, changed = [], False
            for ins in bb.instructions:
                si = ins.sync_info
                if si is not None and si.on_wait is not None and len(si.on_wait) > max_waits:
                    waits = list(si.on_wait)
                    for w in waits[:-max_waits]:
                        out.append(mybir.InstNoOp(
                            name=nc.get_next_instruction_name(),
                            engine=ins.engine, ins=[], outs=[],
                            sync_info=mybir.SyncInfo(on_wait=[w], on_update=[])))
                        n_split += 1
                    ins.sync_info = mybir.SyncInfo(
                        on_wait=waits[-max_waits:], on_update=list(si.on_update))
                    changed = True
                out.append(ins)
            if changed:
                bb.instructions = out
    return n_split


def _build(skv, split_waits=True):
    import concourse.bass as bass
    import concourse.mybir as mybir
    import concourse.tile as tile

    f32 = mybir.dt.float32
    f32r = mybir.dt.float32r
    f16 = mybir.dt.float16
    bf16 = mybir.dt.bfloat16
    Alu = mybir.AluOpType
    Act = mybir.ActivationFunctionType

    nsk = skv // 128

    nc = bass.Bass()
    xqT = nc.declare_dram_parameter("xqT", [E, S], f16, isOutput=False)
    xkT = nc.declare_dram_parameter("xkT", [E, skv], f16, isOutput=False)
    xvT = nc.declare_dram_parameter("xvT", [E, skv], f16, isOutput=False)
    wqT = nc.declare_dram_parameter("wqT", [E, DC], f16, isOutput=False)
    wkT = nc.declare_dram_parameter("wkT", [E, DC], f16, isOutput=False)
    wvT = nc.declare_dram_parameter("wvT", [E, DC], f16, isOutput=False)
    woT = nc.declare_dram_parameter("woT", [DC, E], f16, isOutput=False)
    bq_d = nc.declare_dram_parameter("bq", [DC], f32, isOutput=False)
    bk_d = nc.declare_dram_parameter("bk", [DC], f32, isOutput=False)
    bv_d = nc.declare_dram_parameter("bv", [DC], f32, isOutput=False)
    vm_d = nc.declare_dram_parameter("vmask", [skv], f32, isOutput=False)
    ones_d = nc.declare_dram_parameter("ones64", [1, 64], f32r, isOutput=False)
    out_d = nc.declare_dram_parameter("out", [S, E], f16, isOutput=True)

    xqT_r = xqT.rearrange("(ko p) s -> p ko s", p=128)
    xkT_r = xkT.rearrange("(ko p) s -> p ko s", p=128)
    xvT_r = xvT.rearrange("(ko p) s -> p ko s", p=128)

    with tile.TileContext(nc) as tc:
        with (
            tc.tile_pool(name="weights", bufs=1) as wpool,
            tc.tile_pool(name="consts", bufs=1) as cpool,
            tc.tile_pool(name="persist", bufs=1) as ppool,
            tc.tile_pool(name="xq_s", bufs=2) as xqpool,
            tc.tile_pool(name="xk_s", bufs=2) as xkpool,
            tc.tile_pool(name="xv_s", bufs=9) as xvpool,
            tc.tile_pool(name="et", bufs=2) as etpool,
            tc.tile_pool(name="oun", bufs=3) as oupool,
            tc.tile_pool(name="rc2", bufs=3) as rcpool,
            tc.tile_pool(name="of16", bufs=2) as ofpool,
            tc.tile_pool(name="ob", bufs=3) as obpool,
            tc.tile_pool(name="sc_ps", bufs=2, space="PSUM") as aps,
            tc.tile_pool(name="av_ps", bufs=2, space="PSUM") as avps,
            tc.tile_pool(name="work_ps", bufs=2, space="PSUM") as wps,
        ):
            # ---- constants + weights (DMA order = urgency order) ----
            bq_t = cpool.tile([128, 2], f32, tag="bq")
            bk_t = cpool.tile([128, 2], f32, tag="bk")
            bv_t = cpool.tile([128, DC], f32, tag="bv")
            vm_t = cpool.tile([128, nsk], f32, tag="vm")
            ones_t = cpool.tile([1, 64], f32r, tag="ones")

            wq_t = wpool.tile([128, NB, DC], f16, tag="wq")
            wk_t = wpool.tile([128, NB, DC], f16, tag="wk")
            wv_t = wpool.tile([128, NB, DC], f16, tag="wv")
            wo_t = wpool.tile([128, DC // 128, E], f16, tag="wo")

            qpT = ppool.tile([128, 2, S], f32r, tag="qpT")
            kpT = ppool.tile([128, 2, skv], f32r, tag="kpT")
            vp = ppool.tile([128, nsk, HPG * (D + 1)], bf16, tag="vp")

            def proj_q(nb):
                xq = xqpool.tile([128, NB, QB], f16, tag="xq", name="xq")
                nc.sync.dma_start(xq[:], xqT_r[:, :, nb * QB:(nb + 1) * QB])
                for mc in range(2):
                    ps = wps.tile([128, QB], f32, tag="wp", name="qp_ps")
                    for kc in range(NB):
                        nc.tensor.matmul(
                            ps[:], wq_t[:, kc, mc * 128:(mc + 1) * 128],
                            xq[:, kc, :], start=(kc == 0), stop=(kc == NB - 1))
                    nc.vector.tensor_tensor(
                        out=qpT[:, mc, nb * QB:(nb + 1) * QB], in0=ps[:],
                        in1=bq_t[:, mc:mc + 1].to_broadcast((128, QB)), op=Alu.add)

            def proj_k(sc):
                xk = xkpool.tile([128, NB, 128], f16, tag="xk", name="xk")
                nc.sync.dma_start(xk[:], xkT_r[:, :, sc * 128:(sc + 1) * 128])
                for mc in range(2):
                    ps = wps.tile([128, QB], f32, tag="wp", name="kp_ps")[:, :128]
                    for kc in range(NB):
                        nc.tensor.matmul(
                            ps[:], wk_t[:, kc, mc * 128:(mc + 1) * 128],
                            xk[:, kc, :], start=(kc == 0), stop=(kc == NB - 1))
                    nc.vector.tensor_tensor(
                        out=kpT[:, mc, sc * 128:(sc + 1) * 128], in0=ps[:],
                        in1=bk_t[:, mc:mc + 1].to_broadcast((128, 128)), op=Alu.add)

            xvs = []   # pre-issued xv chunk tiles (DMAs dispatched in lead-in)

            def proj_v(sc):
                xv = xvs[sc]
                ps = wps.tile([128, QB], f32, tag="wp", name="vp_ps")[:, :DC]
                for kc in range(NB):
                    nc.tensor.matmul(
                        ps[:], xv[:, kc, :], wv_t[:, kc, :],
                        start=(kc == 0), stop=(kc == NB - 1))
                t1 = oupool.tile([128, DC], f32, tag="vtmp", name="vtmp")
                nc.vector.tensor_tensor(out=t1[:], in0=ps[:], in1=bv_t[:], op=Alu.add)
                vps = vp[:, sc, :].rearrange("p (h w) -> p h w", w=D + 1)
                nc.gpsimd.tensor_tensor(
                    out=vps[:, :, 0:D],
                    in0=t1.rearrange("p (h w) -> p h w", w=D),
                    in1=vm_t[:, sc:sc + 1, None].to_broadcast((128, HPG, D)),
                    op=Alu.mult)
                nc.gpsimd.tensor_copy(
                    out=vps[:, :, D:D + 1],
                    in_=vm_t[:, sc:sc + 1, None].to_broadcast((128, HPG, 1)))

            # ---- lead-in ----
            # DMA dispatch is spread over SP (input streams) and Act (weights,
            # idle until the first exp) so the SP sequencer (565ns/trigger)
            # doesn't serialize the lead-in. SP order: xq0, consts, xk chunks,
            # xv chunks (prefetched into a 9-deep pool for the proj_v fillers).
            nc.scalar.dma_start(wq_t[:], wqT.rearrange("(ko p) m -> p ko m", p=128))
            nc.scalar.dma_start(wk_t[:], wkT.rearrange("(ko p) m -> p ko m", p=128))
            nc.scalar.dma_start(wv_t[:], wvT.rearrange("(ko p) m -> p ko m", p=128))
            nc.scalar.dma_start(wo_t[:], woT.rearrange("(ko p) n -> p ko n", p=128))
            nc.scalar.dma_start(bq_t[:], bq_d.rearrange("(c p) -> p c", p=128))
            nc.scalar.dma_start(bk_t[:], bk_d.rearrange("(c p) -> p c", p=128))
            nc.scalar.dma_start(bv_t[:], bv_d[None, :].to_broadcast((128, DC)))
            nc.scalar.dma_start(vm_t[:], vm_d.rearrange("(s p) -> p s", p=128))
            nc.scalar.dma_start(ones_t[:], ones_d[:])
            proj_q(0)
            for sc in range(nsk):
                xv = xvpool.tile([128, NB, 128], f16, tag="xv", name=f"xv{sc}")
                nc.sync.dma_start(xv[:], xvT_r[:, :, sc * 128:(sc + 1) * 128])
                xvs.append(xv)

            # ---- main loop ----
            # Software pipeline (unit u = (qb, pt)): scores(u) stream to the
            # Act engine while AV(u-1) matmuls interleave between them at skc
            # granularity, so the PE never parks at an AV waiting on exp(u)
            # and the Act engine never starves. All other PE work (normalize
            # broadcast, fc_out, next-block q projection) is queued as small
            # "filler" closures popped between steps; pops start at step 5 of
            # each unit so the normalize recip DMA round trip (~4us) has
            # settled before its broadcast matmul reaches the PE stream.
            of16 = {}
            ets = {}
            pending_av = []     # (qb, pt)
            filler_q = []

            def pop_filler(n=1):
                for _ in range(n):
                    if filler_q:
                        filler_q.pop(0)()

            def queue_norm(qb, pt, o_unp, rc2s):
                if qb not in of16:
                    of16[qb] = ofpool.tile(
                        [128, 2, QB], f16, tag="of", name=f"of16_{qb}")
                o = of16[qb]

                def mk(j):
                    def go():
                        rc_ps = avps.tile([64, QB], f32, tag="av", name="rc_ps")
                        nc.tensor.matmul(
                            rc_ps[:], ones_t[:], rc2s[0:1, j, :],
                            start=True, stop=True, skip_group_check=True)
                        nc.vector.tensor_tensor(
                            out=o[64 * j:64 * j + 64, pt, :],
                            in0=o_unp[64 * j:64 * j + 64, :], in1=rc_ps[:],
                            op=Alu.mult)
                    return go
                filler_q.append(mk(0))
                filler_q.append(mk(1))

            def queue_fc(qb, on_act=False):
                o = of16.pop(qb)
                obs = {}

                def mk(sqc, eb):
                    def go():
                        if sqc not in obs:
                            obs[sqc] = obpool.tile(
                                [128, 2, QB], f16, tag="ob", name="ob")
                        fps = wps.tile([128, QB], f32, tag="wp", name="fc_ps")
                        nc.tensor.matmul(
                            fps[:], o[:, 0, sqc * 128:(sqc + 1) * 128],
                            wo_t[:, 0, eb * QB:(eb + 1) * QB],
                            start=True, stop=False, skip_group_check=True)
                        nc.tensor.matmul(
                            fps[:], o[:, 1, sqc * 128:(sqc + 1) * 128],
                            wo_t[:, 1, eb * QB:(eb + 1) * QB],
                            start=False, stop=True, skip_group_check=True)
                        if on_act:   # Act is idle after the last exp
                            nc.scalar.copy(out=obs[sqc][:, eb, :], in_=fps[:])
                        else:
                            nc.vector.tensor_copy(
                                out=obs[sqc][:, eb, :], in_=fps[:])
                        if eb == 1:
                            nc.sync.dma_start(
                                out_d[qb * QB + sqc * 128:
                                      qb * QB + (sqc + 1) * 128, :],
                                obs[sqc][:])
                    return go
                for sqc in range(QB // 128):
                    for eb in range(2):
                        filler_q.append(mk(sqc, eb))

            def queue_proj_q(nb):
                xq = xqpool.tile([128, NB, QB], f16, tag="xq", name="xq")
                nc.sync.dma_start(xq[:], xqT_r[:, :, nb * QB:(nb + 1) * QB])
                pss = {}

                def mk(mc):
                    def go():
                        ps = wps.tile([128, QB], f32, tag="wp", name="qp_ps")
                        pss[mc] = ps
                        for kc in range(NB):
                            nc.tensor.matmul(
                                ps[:], wq_t[:, kc, mc * 128:(mc + 1) * 128],
                                xq[:, kc, :], start=(kc == 0),
                                stop=(kc == NB - 1), skip_group_check=True)
                        nc.vector.tensor_tensor(
                            out=qpT[:, mc, nb * QB:(nb + 1) * QB], in0=ps[:],
                            in1=bq_t[:, mc:mc + 1].to_broadcast((128, QB)),
                            op=Alu.add)
                    return go
                filler_q.append(mk(0))
                filler_q.append(mk(1))

            def av_finish(qb, pt, ps_avs, on_act=False):
                """Drain one unit's AV psums: copy dims to SBUF; reciprocal of
                the two sums rows via a partition-packed [128, 8] round trip
                (plain [1, 512] reciprocal costs ~6.5ns/elem = 3.4us)."""
                o_unp = oupool.tile([128, QB], f32, tag="ou", name="o_unp")
                sums2 = rcpool.tile([1, 2, QB], f32, tag="sums", name="sums2")
                for j in range(2):
                    if on_act:   # Act is idle after the last exp
                        nc.scalar.copy(
                            out=o_unp[64 * j:64 * j + 64, :],
                            in_=ps_avs[j][0:D, :])
                        nc.scalar.copy(
                            out=sums2[0:1, j, :], in_=ps_avs[j][D:D + 1, :])
                        continue
                    nc.vector.tensor_copy(
                        out=o_unp[64 * j:64 * j + 64, :], in_=ps_avs[j][0:D, :])
                    nc.vector.tensor_copy(
                        out=sums2[0:1, j, :], in_=ps_avs[j][D:D + 1, :])
                rcT = rcpool.tile([128, 2 * QB // 128], f32, tag="rcT", name="rcT")
                nc.sync.dma_start(rcT[:], sums2[0:1, :, :])
                rcT2 = rcpool.tile([128, 2 * QB // 128], f32r, tag="rcT2",
                                   name="rcT2")
                with nc.allow_low_precision(
                        reason="softmax denom recip as f32r matmul rhs"):
                    nc.vector.reciprocal(out=rcT2[:], in_=rcT[:])
                rc2s = rcpool.tile([1, 2, QB], f32r, tag="rc2s", name="rc2s")
                nc.sync.dma_start(rc2s[0:1, :, :], rcT2[:])
                queue_norm(qb, pt, o_unp, rc2s)

            def unit_steps(qb, pt, et, prev, pet, ps_avs, fill_from=5, fill_n=2,
                           with_proj_k=False):
                for skc in range(nsk):
                    # pop fillers at step START: a proj_v filler must be
                    # emitted before the AV matmul that reads its vp chunk
                    if skc >= fill_from:
                        pop_filler(fill_n)
                    if with_proj_k:
                        # unit u0: K projection chunk feeds this step's scores
                        proj_k(skc)
                    if et is not None:
                        psx = aps.tile([128, 2, QB], f32, tag="sc", name="psx")
                        for j in range(2):
                            nc.tensor.matmul(
                                psx[:, j, :],
                                kpT[64 * j:64 * j + 64, pt,
                                    skc * 128:(skc + 1) * 128],
                                qpT[64 * j:64 * j + 64, pt,
                                    qb * QB:(qb + 1) * QB],
                                start=True, stop=True, tile_position=(64 * j, 0))
                        nc.scalar.activation(et[:, skc, :, :], psx[:], Act.Exp)
                    if prev is not None:
                        pqb, ppt = prev
                        for j in range(2):
                            hl = 2 * ppt + j
                            nc.tensor.matmul(
                                ps_avs[j][:],
                                vp[:, skc, hl * (D + 1):(hl + 1) * (D + 1)],
                                pet[:, skc, j, :],
                                start=(skc == 0), stop=(skc == nsk - 1),
                                skip_group_check=True)

            # V projection runs as fillers during unit u1 (its xv DMAs land
            # behind the xk stream; vp chunk c is ready just ahead of the
            # interleaved AV(u0) matmul that consumes it).
            for sc in range(nsk):
                filler_q.append(lambda sc=sc: proj_v(sc))

            units = [(qb, pt) for qb in range(NQB) for pt in range(2)]
            for ui, (qb, pt) in enumerate(units):
                et = etpool.tile([128, nsk, 2, QB], bf16, tag="et", name="et")
                ets[(qb, pt)] = et
                prev = pending_av.pop(0) if pending_av else None
                pet = ps_avs = None
                if prev is not None:
                    pet = ets.pop(prev)
                    ps_avs = [avps.tile([D + 1, QB], f32, tag="av",
                                        name=f"ps_av{j}") for j in range(2)]
                if ui == 0:
                    # u0: K-projection chunks pace the score stream directly
                    unit_steps(qb, pt, et, None, None, None,
                               fill_from=nsk, with_proj_k=True)
                elif ui == 1:
                    # u1: V-projection fillers (2/step from step 0)
                    unit_steps(qb, pt, et, prev, pet, ps_avs, fill_from=0)
                else:
                    unit_steps(qb, pt, et, prev, pet, ps_avs)
                if prev is not None:
                    av_finish(prev[0], prev[1], ps_avs)
                pending_av.append((qb, pt))
                if pt == 0 and qb + 1 < NQB:
                    queue_proj_q(qb + 1)
                if pt == 0 and qb >= 1:
                    queue_fc(qb - 1)
            # drain: AV for the last unit with fillers, then final norm + fc.
            # Copies/casts go to the Act engine (idle after the last exp) so
            # the DVE doesn't serialize the tail.
            prev = pending_av.pop(0)
            pet = ets.pop(prev)
            ps_avs = [avps.tile([D + 1, QB], f32, tag="av", name=f"ps_av{j}")
                      for j in range(2)]
            unit_steps(None, None, None, prev, pet, ps_avs)
            av_finish(prev[0], prev[1], ps_avs, on_act=True)
            queue_fc(NQB - 1, on_act=True)
            pop_filler(len(filler_q))

    if split_waits:
        _split_excess_waits(nc)
    return nc


def _prep_inputs(q, k, v, mask, W_qkv, b_qkv, W_out, b_out):
    """Host-side shard/layout prep. Returns (skv, in_maps)."""
    q = np.asarray(q, dtype=np.float32)
    k = np.asarray(k, dtype=np.float32)
    v = np.asarray(v, dtype=np.float32)
    mask = np.asarray(mask)
    W_qkv = np.asarray(W_qkv, dtype=np.float32)
    b_qkv = np.asarray(b_qkv, dtype=np.float32)
    W_out = np.asarray(W_out, dtype=np.float32)

    valid = [np.nonzero(mask[b, 0, 0] != 0)[0] for b in range(B)]
    cnts = [len(vi) for vi in valid]
    skv = max(128, max((c + 127) // 128 * 128 for c in cnts))

    # per-batch tensors
    qT, kTc, vTc, vms = [], [], [], []
    for b in range(B):
        qT.append(np.ascontiguousarray(q[b].T).astype(np.float16))
        kt = np.zeros((E, skv), np.float16)
        vt = np.zeros((E, skv), np.float16)
        kt[:, :cnts[b]] = k[b][valid[b]].T
        vt[:, :cnts[b]] = v[b][valid[b]].T
        kTc.append(kt)
        vTc.append(vt)
        vm = np.zeros((skv,), np.float32)
        vm[:cnts[b]] = 1.0
        vms.append(vm)

    in_maps = []
    for c in range(NCORES):
        b, g = divmod(c, GROUPS)
        sl = slice(g * DC, (g + 1) * DC)
        in_maps.append({
            "xqT": qT[b], "xkT": kTc[b], "xvT": vTc[b],
            "wqT": np.ascontiguousarray(W_qkv[sl, :].T).astype(np.float16),
            "wkT": np.ascontiguousarray(W_qkv[E:][sl, :].T).astype(np.float16),
            "wvT": np.ascontiguousarray(W_qkv[2 * E:][sl, :].T).astype(np.float16),
            "woT": np.ascontiguousarray(W_out[:, sl].T).astype(np.float16),
            "bq": np.ascontiguousarray(b_qkv[sl]),
            "bk": np.ascontiguousarray(b_qkv[E:][sl]),
            "bv": np.ascontiguousarray(b_qkv[2 * E:][sl]),
            "vmask": vms[b],
            "ones64": np.ones((1, 64), np.float32),
        })
    return skv, in_maps


def kernel(q, k, v, mask, W_qkv, b_qkv, W_out, b_out):
    from concourse import bass_utils

    skv, in_maps = _prep_inputs(q, k, v, mask, W_qkv, b_qkv, W_out, b_out)
    if skv not in _CACHE:
        _CACHE[skv] = _build(skv)
    nc = _CACHE[skv]

    trace = os.environ.get("KERNEL_TRACE") == "1"
    if trace:
        bass_utils.upload_artifacts = lambda tmpdir: "local://" + tmpdir
    res = bass_utils.run_bass_kernel_spmd(
        nc, in_maps, list(range(NCORES)), trace=trace)
    if trace:
        print(f"HW exec time: {res.exec_time_ns} ns")
        if res.instructions_and_trace is not None:
            print(f"trace path: {res.instructions_and_trace[1]}")

    b_out = np.asarray(b_out, dtype=np.float32)
    out = np.zeros((B, S, E), np.float32)
    for c in range(NCORES):
        out[c // GROUPS] += res.results[c]["out"].astype(np.float32)
    out += b_out[None, None, :]
    return out


# revision 38
# speedup vs baseline: 1.0314x; 1.0314x over previous
"""Multi-head attention (B=2, S=2048, E=1024, H=16) on 8 TRN2 NeuronCores.

Sharding: batch x head-group. Core c handles batch b=c//4 and head group
g=c%4 (4 heads = 256 of E). Each core computes its heads' attention output
slice and a partial fc_out product [S, E]; the host sums the 4 partials per
batch and adds b_out.

v2 design notes (vs the 239us baseline):
- All HBM input tensors are f16 (host-cast); qpT/kpT kept on-chip in f32r.
- Single fused [S, E] f16 output per core: fc_out contracts K=256 over both
  head-pairs (2 accumulating matmuls) -> half the output DMA and copies.
- Loop order: outer qb (512-query block), inner pt (head pair). Normalize
  and fc_out are software-pipelined one unit behind attention so the PE
  instruction stream never waits on the recip chain (keeps PE HAM-warm).
- Act engine runs ONLY exp, as [128, 2, 512] pair-instructions spanning the
  two score psum banks. Copies are on gpsimd, recip/normalize-mult on DVE.
- Softmax denominator: ones-column trick in the AV matmul (row 64 of the
  [65, 512] psum); per-query reciprocal row is partition-broadcast with a
  single K=2 matmul against a constant [2, 128] "eye64" block matrix.
- K/V projections are chunked at 128 keys to pace with their DMAs, so the
  PE starts working ~4us in and stays continuously busy.

Mask handling is exact: masked K/V rows are removed on the host (gather),
so softmax(where(mask==0, -1e20, e)) == exp(e_valid)/sum(exp(e_valid)).
"""

import os

import numpy as np

B, S, E, H = 2, 2048, 1024, 16
D = E // H           # 64
NCORES = 8
GROUPS = 4           # head groups per batch (cores per batch)
HPG = H // GROUPS    # 4 heads per core
DC = E // GROUPS     # 256 dims per core
NB = E // 128        # 8 contraction chunks over E
QB = 512             # query block width
NQB = S // QB        # 4

_CACHE = {}


def _split_excess_waits(nc, max_waits=1):
    """walrus rejects instructions carrying >1 sem wait; spread extras onto
    single-wait NoOps inserted before the instruction on the same engine."""
    import concourse.mybir as mybir

    n_split = 0
    for f in nc.m.functions:
        for bb in f.blocks:
            out, changed = [], False
            for ins in bb.instructions:
                si = ins.sync_info
                if si is not None and si.on_wait is not None and len(si.on_wait) > max_waits:
                    waits = list(si.on_wait)
                    for w in waits[:-max_waits]:
                        out.append(mybir.InstNoOp(
                            name=nc.get_next_instruction_name(),
                            engine=ins.engine, ins=[], outs=[],
                            sync_info=mybir.SyncInfo(on_wait=[w], on_update=[])))
                        n_split += 1
                    ins.sync_info = mybir.SyncInfo(
                        on_wait=waits[-max_waits:], on_update=list(si.on_update))
                    changed = True
                out.append(ins)
            if changed:
                bb.instructions = out
    return n_split


def _build(skv, split_waits=True):
    import concourse.bass as bass
    import concourse.mybir as mybir
    import concourse.tile as tile

    f32 = mybir.dt.float32
    f32r = mybir.dt.float32r
    f16 = mybir.dt.float16
    bf16 = mybir.dt.bfloat16
    Alu = mybir.AluOpType
    Act = mybir.ActivationFunctionType

    nsk = skv // 128

    nc = bass.Bass()
    xqT = nc.declare_dram_parameter("xqT", [E, S], f16, isOutput=False)
    xkT = nc.declare_dram_parameter("xkT", [E, skv], f16, isOutput=False)
    xvT = nc.declare_dram_parameter("xvT", [E, skv], f16, isOutput=False)
    wqT = nc.declare_dram_parameter("wqT", [E, DC], f16, isOutput=False)
    wkT = nc.declare_dram_parameter("wkT", [E, DC], f16, isOutput=False)
    wvT = nc.declare_dram_parameter("wvT", [E, DC], f16, isOutput=False)
    woT = nc.declare_dram_parameter("woT", [DC, E], f16, isOutput=False)
    bq_d = nc.declare_dram_parameter("bq", [DC], f32, isOutput=False)
    bk_d = nc.declare_dram_parameter("bk", [DC], f32, isOutput=False)
    bv_d = nc.declare_dram_parameter("bv", [DC], f32, isOutput=False)
    vm_d = nc.declare_dram_parameter("vmask", [skv], f32, isOutput=False)
    ones_d = nc.declare_dram_parameter("ones64", [1, 64], f32r, isOutput=False)
    out_d = nc.declare_dram_parameter("out", [S, E], f16, isOutput=True)

    xqT_r = xqT.rearrange("(ko p) s -> p ko s", p=128)
    xkT_r = xkT.rearrange("(ko p) s -> p ko s", p=128)
    xvT_r = xvT.rearrange("(ko p) s -> p ko s", p=128)

    with tile.TileContext(nc) as tc:
        with (
            tc.tile_pool(name="weights", bufs=1) as wpool,
            tc.tile_pool(name="consts", bufs=1) as cpool,
            tc.tile_pool(name="persist", bufs=1) as ppool,
            tc.tile_pool(name="xq_s", bufs=2) as xqpool,
            tc.tile_pool(name="xk_s", bufs=2) as xkpool,
            tc.tile_pool(name="xv_s", bufs=9) as xvpool,
            tc.tile_pool(name="et", bufs=2) as etpool,
            tc.tile_pool(name="oun", bufs=3) as oupool,
            tc.tile_pool(name="rc2", bufs=3) as rcpool,
            tc.tile_pool(name="of16", bufs=2) as ofpool,
            tc.tile_pool(name="ob", bufs=3) as obpool,
            tc.tile_pool(name="sc_ps", bufs=2, space="PSUM") as aps,
            tc.tile_pool(name="av_ps", bufs=2, space="PSUM") as avps,
            tc.tile_pool(name="work_ps", bufs=2, space="PSUM") as wps,
        ):
            # ---- constants + weights (DMA order = urgency order) ----
            bq_t = cpool.tile([128, 2], f32, tag="bq")
            bk_t = cpool.tile([128, 2], f32, tag="bk")
            bv_t = cpool.tile([128, DC], f32, tag="bv")
            vm_t = cpool.tile([128, nsk], f32, tag="vm")
            ones_t = cpool.tile([1, 64], f32r, tag="ones")

            wq_t = wpool.tile([128, NB, DC], f16, tag="wq")
            wk_t = wpool.tile([128, NB, DC], f16, tag="wk")
            wv_t = wpool.tile([128, NB, DC], f16, tag="wv")
            wo_t = wpool.tile([128, DC // 128, E], f16, tag="wo")

            qpT = ppool.tile([128, 2, S], f32r, tag="qpT")
            kpT = ppool.tile([128, 2, skv], f32r, tag="kpT")
            vp = ppool.tile([128, nsk, HPG * (D + 1)], bf16, tag="vp")

            def proj_q(nb):
                xq = xqpool.tile([128, NB, QB], f16, tag="xq", name="xq")
                nc.sync.dma_start(xq[:], xqT_r[:, :, nb * QB:(nb + 1) * QB])
                for mc in range(2):
                    ps = wps.tile([128, QB], f32, tag="wp", name="qp_ps")
                    for kc in range(NB):
                        nc.tensor.matmul(
                            ps[:], wq_t[:, kc, mc * 128:(mc + 1) * 128],
                            xq[:, kc, :], start=(kc == 0), stop=(kc == NB - 1))
                    nc.vector.tensor_tensor(
                        out=qpT[:, mc, nb * QB:(nb + 1) * QB], in0=ps[:],
                        in1=bq_t[:, mc:mc + 1].to_broadcast((128, QB)), op=Alu.add)

            def proj_k(sc):
                xk = xkpool.tile([128, NB, 128], f16, tag="xk", name="xk")
                nc.sync.dma_start(xk[:], xkT_r[:, :, sc * 128:(sc + 1) * 128])
                for mc in range(2):
                    ps = wps.tile([128, QB], f32, tag="wp", name="kp_ps")[:, :128]
                    for kc in range(NB):
                        nc.tensor.matmul(
                            ps[:], wk_t[:, kc, mc * 128:(mc + 1) * 128],
                            xk[:, kc, :], start=(kc == 0), stop=(kc == NB - 1))
                    nc.vector.tensor_tensor(
                        out=kpT[:, mc, sc * 128:(sc + 1) * 128], in0=ps[:],
                        in1=bk_t[:, mc:mc + 1].to_broadcast((128, 128)), op=Alu.add)

            xvs = []   # pre-issued xv chunk tiles (DMAs dispatched in lead-in)

            def proj_v(sc):
                xv = xvs[sc]
                ps = wps.tile([128, QB], f32, tag="wp", name="vp_ps")[:, :DC]
                for kc in range(NB):
                    nc.tensor.matmul(
                        ps[:], xv[:, kc, :], wv_t[:, kc, :],
                        start=(kc == 0), stop=(kc == NB - 1))
                t1 = oupool.tile([128, DC], f32, tag="vtmp", name="vtmp")
                nc.vector.tensor_tensor(out=t1[:], in0=ps[:], in1=bv_t[:], op=Alu.add)
                vps = vp[:, sc, :].rearrange("p (h w) -> p h w", w=D + 1)
                nc.gpsimd.tensor_tensor(
                    out=vps[:, :, 0:D],
                    in0=t1.rearrange("p (h w) -> p h w", w=D),
                    in1=vm_t[:, sc:sc + 1, None].to_broadcast((128, HPG, D)),
                    op=Alu.mult)
                nc.gpsimd.tensor_copy(
                    out=vps[:, :, D:D + 1],
                    in_=vm_t[:, sc:sc + 1, None].to_broadcast((128, HPG, 1)))

            # ---- lead-in ----
            # DMA dispatch is spread over SP (input streams) and Act (weights,
            # idle until the first exp) so the SP sequencer (565ns/trigger)
            # doesn't serialize the lead-in. SP order: xq0, consts, xk chunks,
            # xv chunks (prefetched into a 9-deep pool for the proj_v fillers).
            nc.scalar.dma_start(wq_t[:], wqT.rearrange("(ko p) m -> p ko m", p=128))
            nc.scalar.dma_start(wk_t[:], wkT.rearrange("(ko p) m -> p ko m", p=128))
            nc.scalar.dma_start(wv_t[:], wvT.rearrange("(ko p) m -> p ko m", p=128))
            nc.scalar.dma_start(wo_t[:], woT.rearrange("(ko p) n -> p ko n", p=128))
            nc.scalar.dma_start(bq_t[:], bq_d.rearrange("(c p) -> p c", p=128))
            nc.scalar.dma_start(bk_t[:], bk_d.rearrange("(c p) -> p c", p=128))
            nc.scalar.dma_start(bv_t[:], bv_d[None, :].to_broadcast((128, DC)))
            nc.scalar.dma_start(vm_t[:], vm_d.rearrange("(s p) -> p s", p=128))
            nc.scalar.dma_start(ones_t[:], ones_d[:])
            proj_q(0)
            for sc in range(nsk):
                xv = xvpool.tile([128, NB, 128], f16, tag="xv", name=f"xv{sc}")
                nc.sync.dma_start(xv[:], xvT_r[:, :, sc * 128:(sc + 1) * 128])
                xvs.append(xv)

            # ---- main loop ----
            # Software pipeline (unit u = (qb, pt)): scores(u) stream to the
            # Act engine while AV(u-1) matmuls interleave between them at skc
            # granularity, so the PE never parks at an AV waiting on exp(u)
            # and the Act engine never starves. All other PE work (normalize
            # broadcast, fc_out, next-block q projection) is queued as small
            # "filler" closures popped between steps; pops start at step 5 of
            # each unit so the normalize recip DMA round trip (~4us) has
            # settled before its broadcast matmul reaches the PE stream.
            of16 = {}
            ets = {}
            pending_av = []     # (qb, pt)
            filler_q = []

            def pop_filler(n=1):
                for _ in range(n):
                    if filler_q:
                        filler_q.pop(0)()

            def queue_norm(qb, pt, o_unp, rc2s):
                if qb not in of16:
                    of16[qb] = ofpool.tile(
                        [128, 2, QB], f16, tag="of", name=f"of16_{qb}")
                o = of16[qb]

                def mk(j):
                    def go():
                        rc_ps = avps.tile([64, QB], f32, tag="av", name="rc_ps")
                        nc.tensor.matmul(
                            rc_ps[:], ones_t[:], rc2s[0:1, j, :],
                            start=True, stop=True, skip_group_check=True)
                        nc.vector.tensor_tensor(
                            out=o[64 * j:64 * j + 64, pt, :],
                            in0=o_unp[64 * j:64 * j + 64, :], in1=rc_ps[:],
                            op=Alu.mult)
                    return go
                filler_q.append(mk(0))
                filler_q.append(mk(1))

            def queue_fc(qb, on_act=False):
                o = of16.pop(qb)
                obs = {}

                def mk(sqc, eb):
                    def go():
                        if sqc not in obs:
                            obs[sqc] = obpool.tile(
                                [128, 2, QB], f16, tag="ob", name="ob")
                        fps = wps.tile([128, QB], f32, tag="wp", name="fc_ps")
                        nc.tensor.matmul(
                            fps[:], o[:, 0, sqc * 128:(sqc + 1) * 128],
                            wo_t[:, 0, eb * QB:(eb + 1) * QB],
                            start=True, stop=False, skip_group_check=True)
                        nc.tensor.matmul(
                            fps[:], o[:, 1, sqc * 128:(sqc + 1) * 128],
                            wo_t[:, 1, eb * QB:(eb + 1) * QB],
                            start=False, stop=True, skip_group_check=True)
                        if on_act:   # Act is idle after the last exp
                            nc.scalar.copy(out=obs[sqc][:, eb, :], in_=fps[:])
                        else:
                            nc.vector.tensor_copy(
                                out=obs[sqc][:, eb, :], in_=fps[:])
                        if eb == 1:
                            nc.sync.dma_start(
                                out_d[qb * QB + sqc * 128:
                                      qb * QB + (sqc + 1) * 128, :],
                                obs[sqc][:])
                    return go
                for sqc in range(QB // 128):
                    for eb in range(2):
                        filler_q.append(mk(sqc, eb))

            def queue_proj_q(nb):
                xq = xqpool.tile([128, NB, QB], f16, tag="xq", name="xq")
                nc.sync.dma_start(xq[:], xqT_r[:, :, nb * QB:(nb + 1) * QB])
                pss = {}

                def mk(mc):
                    def go():
                        ps = wps.tile([128, QB], f32, tag="wp", name="qp_ps")
                        pss[mc] = ps
                        for kc in range(NB):
                            nc.tensor.matmul(
                                ps[:], wq_t[:, kc, mc * 128:(mc + 1) * 128],
                                xq[:, kc, :], start=(kc == 0),
                                stop=(kc == NB - 1), skip_group_check=True)
                        nc.vector.tensor_tensor(
                            out=qpT[:, mc, nb * QB:(nb + 1) * QB], in0=ps[:],
                            in1=bq_t[:, mc:mc + 1].to_broadcast((128, QB)),
                            op=Alu.add)
                    return go
                filler_q.append(mk(0))
                filler_q.append(mk(1))

            def av_finish(qb, pt, ps_avs, on_act=False):
                """Drain one unit's AV psums: copy dims to SBUF; reciprocal of
                the two sums rows via a partition-packed [128, 8] round trip
                (plain [1, 512] reciprocal costs ~6.5ns/elem = 3.4us)."""
                o_unp = oupool.tile([128, QB], f32, tag="ou", name="o_unp")
                sums2 = rcpool.tile([1, 2, QB], f32, tag="sums", name="sums2")
                for j in range(2):
                    if on_act:   # Act is idle after the last exp
                        nc.scalar.copy(
                            out=o_unp[64 * j:64 * j + 64, :],
                            in_=ps_avs[j][0:D, :])
                        nc.scalar.copy(
                            out=sums2[0:1, j, :], in_=ps_avs[j][D:D + 1, :])
                        continue
                    nc.vector.tensor_copy(
                        out=o_unp[64 * j:64 * j + 64, :], in_=ps_avs[j][0:D, :])
                    nc.vector.tensor_copy(
                        out=sums2[0:1, j, :], in_=ps_avs[j][D:D + 1, :])
                rcT = rcpool.tile([128, 2 * QB // 128], f32, tag="rcT", name="rcT")
                nc.sync.dma_start(rcT[:], sums2[0:1, :, :])
                rcT2 = rcpool.tile([128, 2 * QB // 128], f32r, tag="rcT2",
                                   name="rcT2")
                with nc.allow_low_precision(
                        reason="softmax denom recip as f32r matmul rhs"):
                    nc.vector.reciprocal(out=rcT2[:], in_=rcT[:])
                rc2s = rcpool.tile([1, 2, QB], f32r, tag="rc2s", name="rc2s")
                nc.sync.dma_start(rc2s[0:1, :, :], rcT2[:])
                queue_norm(qb, pt, o_unp, rc2s)

            def unit_steps(qb, pt, et, prev, pet, ps_avs, fill_from=5, fill_n=2,
                           with_proj_k=False):
                for skc in range(nsk):
                    if with_proj_k:
                        # unit u0: K projection chunk feeds this step's scores
                        proj_k(skc)
                    if et is not None:
                        psx = aps.tile([128, 2, QB], f32, tag="sc", name="psx")
                        for j in range(2):
                            nc.tensor.matmul(
                                psx[:, j, :],
                                kpT[64 * j:64 * j + 64, pt,
                                    skc * 128:(skc + 1) * 128],
                                qpT[64 * j:64 * j + 64, pt,
                                    qb * QB:(qb + 1) * QB],
                                start=True, stop=True, tile_position=(64 * j, 0))
                        nc.scalar.activation(et[:, skc, :, :], psx[:], Act.Exp)
                    # pops sit between the score pair (keeps Act fed) and the
                    # AV matmuls (a proj_v filler must be emitted before the
                    # AV matmul that reads its vp chunk)
                    if skc >= fill_from:
                        pop_filler(fill_n)
                    if prev is not None:
                        pqb, ppt = prev
                        for j in range(2):
                            hl = 2 * ppt + j
                            nc.tensor.matmul(
                                ps_avs[j][:],
                                vp[:, skc, hl * (D + 1):(hl + 1) * (D + 1)],
                                pet[:, skc, j, :],
                                start=(skc == 0), stop=(skc == nsk - 1),
                                skip_group_check=True)

            # V projection runs as fillers during unit u1 (its xv DMAs land
            # behind the xk stream; vp chunk c is ready just ahead of the
            # interleaved AV(u0) matmul that consumes it).
            for sc in range(nsk):
                filler_q.append(lambda sc=sc: proj_v(sc))

            units = [(qb, pt) for qb in range(NQB) for pt in range(2)]
            for ui, (qb, pt) in enumerate(units):
                et = etpool.tile([128, nsk, 2, QB], bf16, tag="et", name="et")
                ets[(qb, pt)] = et
                prev = pending_av.pop(0) if pending_av else None
                pet = ps_avs = None
                if prev is not None:
                    pet = ets.pop(prev)
                    ps_avs = [avps.tile([D + 1, QB], f32, tag="av",
                                        name=f"ps_av{j}") for j in range(2)]
                if ui == 0:
                    # u0: K-projection chunks pace the score stream directly
                    unit_steps(qb, pt, et, None, None, None,
                               fill_from=nsk, with_proj_k=True)
                elif ui == 1:
                    # u1: V-projection fillers (2/step from step 0)
                    unit_steps(qb, pt, et, prev, pet, ps_avs, fill_from=0)
                else:
                    unit_steps(qb, pt, et, prev, pet, ps_avs)
                if prev is not None:
                    av_finish(prev[0], prev[1], ps_avs)
                pending_av.append((qb, pt))
                if pt == 0 and qb + 1 < NQB:
                    queue_proj_q(qb + 1)
                if pt == 0 and qb >= 1:
                    queue_fc(qb - 1)
            # drain: AV for the last unit with fillers, then final norm + fc.
            # Copies/casts go to the Act engine (idle after the last exp) so
            # the DVE doesn't serialize the tail.
            prev = pending_av.pop(0)
            pet = ets.pop(prev)
            ps_avs = [avps.tile([D + 1, QB], f32, tag="av", name=f"ps_av{j}")
                      for j in range(2)]
            unit_steps(None, None, None, prev, pet, ps_avs)
            av_finish(prev[0], prev[1], ps_avs, on_act=True)
            queue_fc(NQB - 1, on_act=True)
            pop_filler(len(filler_q))

    if split_waits:
        _split_excess_waits(nc)
    return nc


def _prep_inputs(q, k, v, mask, W_qkv, b_qkv, W_out, b_out):
    """Host-side shard/layout prep. Returns (skv, in_maps)."""
    q = np.asarray(q, dtype=np.float32)
    k = np.asarray(k, dtype=np.float32)
    v = np.asarray(v, dtype=np.float32)
    mask = np.asarray(mask)
    W_qkv = np.asarray(W_qkv, dtype=np.float32)
    b_qkv = np.asarray(b_qkv, dtype=np.float32)
    W_out = np.asarray(W_out, dtype=np.float32)

    valid = [np.nonzero(mask[b, 0, 0] != 0)[0] for b in range(B)]
    cnts = [len(vi) for vi in valid]
    skv = max(128, max((c + 127) // 128 * 128 for c in cnts))

    # per-batch tensors
    qT, kTc, vTc, vms = [], [], [], []
    for b in range(B):
        qT.append(np.ascontiguousarray(q[b].T).astype(np.float16))
        kt = np.zeros((E, skv), np.float16)
        vt = np.zeros((E, skv), np.float16)
        kt[:, :cnts[b]] = k[b][valid[b]].T
        vt[:, :cnts[b]] = v[b][valid[b]].T
        kTc.append(kt)
        vTc.append(vt)
        vm = np.zeros((skv,), np.float32)
        vm[:cnts[b]] = 1.0
        vms.append(vm)

    in_maps = []
    for c in range(NCORES):
        b, g = divmod(c, GROUPS)
        sl = slice(g * DC, (g + 1) * DC)
        in_maps.append({
            "xqT": qT[b], "xkT": kTc[b], "xvT": vTc[b],
            "wqT": np.ascontiguousarray(W_qkv[sl, :].T).astype(np.float16),
            "wkT": np.ascontiguousarray(W_qkv[E:][sl, :].T).astype(np.float16),
            "wvT": np.ascontiguousarray(W_qkv[2 * E:][sl, :].T).astype(np.float16),
            "woT": np.ascontiguousarray(W_out[:, sl].T).astype(np.float16),
            "bq": np.ascontiguousarray(b_qkv[sl]),
            "bk": np.ascontiguousarray(b_qkv[E:][sl]),
            "bv": np.ascontiguousarray(b_qkv[2 * E:][sl]),
            "vmask": vms[b],
            "ones64": np.ones((1, 64), np.float32),
        })
    return skv, in_maps


def kernel(q, k, v, mask, W_qkv, b_qkv, W_out, b_out):
    from concourse import bass_utils

    skv, in_maps = _prep_inputs(q, k, v, mask, W_qkv, b_qkv, W_out, b_out)
    if skv not in _CACHE:
        _CACHE[skv] = _build(skv)
    nc = _CACHE[skv]

    trace = os.environ.get("KERNEL_TRACE") == "1"
    if trace:
        bass_utils.upload_artifacts = lambda tmpdir: "local://" + tmpdir
    res = bass_utils.run_bass_kernel_spmd(
        nc, in_maps, list(range(NCORES)), trace=trace)
    if trace:
        print(f"HW exec time: {res.exec_time_ns} ns")
        if res.instructions_and_trace is not None:
            print(f"trace path: {res.instructions_and_trace[1]}")

    b_out = np.asarray(b_out, dtype=np.float32)
    out = np.zeros((B, S, E), np.float32)
    for c in range(NCORES):
        out[c // GROUPS] += res.results[c]["out"].astype(np.float32)
    out += b_out[None, None, :]
    return out
